# revision 1
# baseline (speedup 1.0000x reference)
"""Data-parallel ActorNetwork kernel for 8 Trainium2 NeuronCores.

Sharding: pure data parallel — batch dim B=16384 split 8 ways (2048 rows
per core); all weights (<1 MB) replicated to every core. Each core runs
the full per-sample network; outputs are concatenated on the host.

Self-contained: hardcodes B=16384, N=32, OBS=(16,64,16), D=64, H=3,
NACT=2 from the problem spec.
"""

import numpy as np
import jax
import jax.numpy as jnp

M = 8  # NeuronCores
B = 16384
N = 32
D = 64
H = 3
NACT = 2

_SQRT_D = np.sqrt(np.float32(D)).astype(np.float32)


def _forward(s0, s1, s2, w):
    """Per-shard forward pass (identical math to the reference)."""
    own_e = jax.nn.relu(s0 @ w["W_own"].T + w["b_own"])        # [b, 64]
    env_e = jax.nn.relu(s1 @ w["W_env"].T + w["b_env"])        # [b, 64]
    h = jax.nn.relu(s2 @ w["W_s1"].T + w["b_s1"])              # [b, N, 64]
    intru_e = jax.nn.relu(h @ w["W_s2"].T + w["b_s2"])         # [b, N, 64]

    q = own_e @ w["W_q"].T                                      # [b, 64]
    k = intru_e @ w["W_k"].T                                    # [b, N, 64]
    v = intru_e @ w["W_v"].T                                    # [b, N, 64]
    mask = jnp.mean(intru_e, axis=2, keepdims=True) != 0        # [b, N, 1]
    score = jnp.einsum("bnd,bd->bn", k, q)[..., None]           # [b, N, 1]
    score = jnp.where(mask, score, -jnp.inf)
    alpha = jax.nn.softmax(score / _SQRT_D, axis=1)
    alpha = jnp.where(mask, alpha, 0.0)
    v_att = jnp.sum(v * alpha, axis=1)                          # [b, 64]

    concat = jnp.concatenate([own_e, env_e, v_att], axis=1)     # [b, 192]

    # Multi-head self-attention over seq_len=1: softmax over a single
    # key is exactly 1.0, so attn @ comV == comV; comQ/comK cancel out.
    b = concat.shape[0]
    raw = concat.reshape(b, 1, H, D)
    comV = (raw @ w["W_cv"].T).transpose(0, 2, 1, 3)            # [b, H, 1, D]
    concat_m = comV.transpose(0, 2, 1, 3).reshape(b, 1, H * D)  # [b, 1, 192]

    mo = concat_m @ w["W_mo"].T + w["b_mo"]                     # [b, 1, 128]
    a = jax.nn.relu(mo @ w["W_a1"].T + w["b_a1"])               # [b, 1, 64]
    return jnp.tanh(a @ w["W_a2"].T + w["b_a2"])                # [b, 1, NACT]


_pmapped = jax.pmap(_forward, in_axes=(0, 0, 0, None))

_compiled = False


def kernel(**inputs) -> np.ndarray:
    global _compiled
    s0 = np.asarray(inputs["s0"], dtype=np.float32).reshape(M, B // M, -1)
    s1 = np.asarray(inputs["s1"], dtype=np.float32).reshape(M, B // M, -1)
    s2 = np.asarray(inputs["s2"], dtype=np.float32).reshape(M, B // M, N, -1)
    w = {
        k: jnp.asarray(np.asarray(v, dtype=np.float32))
        for k, v in inputs.items()
        if k not in ("s0", "s1", "s2")
    }
    out = _pmapped(jnp.asarray(s0), jnp.asarray(s1), jnp.asarray(s2), w)
    out = np.asarray(jax.device_get(out), dtype=np.float32)
    return out.reshape(B, 1, NACT)


# revision 2
# speedup vs baseline: 1.0452x; 1.0452x over previous
"""Data-parallel ActorNetwork kernel for 8 Trainium2 NeuronCores.

Sharding: pure data parallel — batch dim B=16384 split 8 ways (2048 rows
per core); all weights (<1 MB) replicated to every core. Each core runs
the full per-sample network; outputs are concatenated on the host.

Self-contained: hardcodes B=16384, N=32, OBS=(16,64,16), D=64, H=3,
NACT=2 from the problem spec.
"""

import numpy as np
import jax
import jax.numpy as jnp

M = 8  # NeuronCores
B = 16384
N = 32
D = 64
H = 3
NACT = 2

_SQRT_D = np.sqrt(np.float32(D)).astype(np.float32)


def _forward(s0, s1, s2, w):
    """Per-shard forward pass (identical math to the reference)."""
    own_e = jax.nn.relu(s0 @ w["W_own"].T + w["b_own"])        # [b, 64]
    env_e = jax.nn.relu(s1 @ w["W_env"].T + w["b_env"])        # [b, 64]
    h = jax.nn.relu(s2 @ w["W_s1"].T + w["b_s1"])              # [b, N, 64]
    intru_e = jax.nn.relu(h @ w["W_s2"].T + w["b_s2"])         # [b, N, 64]

    # Fold W_k into the query and W_v past the pooling sum: both are exact
    # linear reassociations that keep the [b, N, 64] tensor path free of
    # the two large per-neighbor projections.
    q = own_e @ w["W_q"].T                                      # [b, 64]
    qk = q @ w["W_k"]                                           # [b, 64]
    mask = jnp.mean(intru_e, axis=2, keepdims=True) != 0        # [b, N, 1]
    score = jnp.einsum("bnd,bd->bn", intru_e, qk)[..., None]    # [b, N, 1]
    score = jnp.where(mask, score, -jnp.inf)
    alpha = jax.nn.softmax(score / _SQRT_D, axis=1)
    alpha = jnp.where(mask, alpha, 0.0)
    pooled = jnp.sum(intru_e * alpha, axis=1)                   # [b, 64]
    v_att = pooled @ w["W_v"].T                                 # [b, 64]

    concat = jnp.concatenate([own_e, env_e, v_att], axis=1)     # [b, 192]

    # Multi-head self-attention over seq_len=1: softmax over a single
    # key is exactly 1.0, so attn @ comV == comV; comQ/comK cancel out.
    b = concat.shape[0]
    raw = concat.reshape(b, 1, H, D)
    comV = (raw @ w["W_cv"].T).transpose(0, 2, 1, 3)            # [b, H, 1, D]
    concat_m = comV.transpose(0, 2, 1, 3).reshape(b, 1, H * D)  # [b, 1, 192]

    mo = concat_m @ w["W_mo"].T + w["b_mo"]                     # [b, 1, 128]
    a = jax.nn.relu(mo @ w["W_a1"].T + w["b_a1"])               # [b, 1, 64]
    return jnp.tanh(a @ w["W_a2"].T + w["b_a2"])                # [b, 1, NACT]


_pmapped = jax.pmap(_forward, in_axes=(0, 0, 0, None))

_compiled = False


def kernel(**inputs) -> np.ndarray:
    global _compiled
    s0 = np.asarray(inputs["s0"], dtype=np.float32).reshape(M, B // M, -1)
    s1 = np.asarray(inputs["s1"], dtype=np.float32).reshape(M, B // M, -1)
    s2 = np.asarray(inputs["s2"], dtype=np.float32).reshape(M, B // M, N, -1)
    w = {
        k: jnp.asarray(np.asarray(v, dtype=np.float32))
        for k, v in inputs.items()
        if k not in ("s0", "s1", "s2")
    }
    out = _pmapped(jnp.asarray(s0), jnp.asarray(s1), jnp.asarray(s2), w)
    out = np.asarray(jax.device_get(out), dtype=np.float32)
    return out.reshape(B, 1, NACT)


# revision 3
# speedup vs baseline: 1.0893x; 1.0422x over previous
"""Data-parallel ActorNetwork kernel for 8 Trainium2 NeuronCores.

Sharding: pure data parallel — batch dim B=16384 split 8 ways (2048 rows
per core); all weights (<1 MB) replicated to every core. Each core runs
the full per-sample network; outputs are concatenated on the host.

Self-contained: hardcodes B=16384, N=32, OBS=(16,64,16), D=64, H=3,
NACT=2 from the problem spec.
"""

import numpy as np
import jax
import jax.numpy as jnp

M = 8  # NeuronCores
B = 16384
N = 32
D = 64
H = 3
NACT = 2

_SQRT_D = np.sqrt(np.float32(D)).astype(np.float32)


def _forward(s0, s1, s2, w):
    """Per-shard forward pass (identical math to the reference)."""
    own_e = jax.nn.relu(s0 @ w["W_own"].T + w["b_own"])        # [b, 64]
    env_e = jax.nn.relu(s1 @ w["W_env"].T + w["b_env"])        # [b, 64]
    h = jax.nn.relu(s2 @ w["W_s1"].T + w["b_s1"])              # [b, N, 64]
    intru_e = jax.nn.relu(h @ w["W_s2"].T + w["b_s2"])         # [b, N, 64]

    # Fold W_k into the query and W_v past the pooling sum: both are exact
    # linear reassociations that keep the [b, N, 64] tensor path free of
    # the two large per-neighbor projections.
    q = own_e @ w["W_q"].T                                      # [b, 64]
    qk = q @ w["W_k"]                                           # [b, 64]
    mask = jnp.mean(intru_e, axis=2, keepdims=True) != 0        # [b, N, 1]
    score = jnp.einsum("bnd,bd->bn", intru_e, qk)[..., None]    # [b, N, 1]
    score = jnp.where(mask, score, -jnp.inf)
    alpha = jax.nn.softmax(score / _SQRT_D, axis=1)
    alpha = jnp.where(mask, alpha, 0.0)
    pooled = jnp.sum(intru_e * alpha, axis=1)                   # [b, 64]

    # The seq-len-1 self-attention softmax is identically 1.0, so
    # attn @ comV == comV and comQ/comK cancel. The remaining chain
    # (per-chunk W_cv, then W_mo, with W_v ahead of it on the pooled
    # branch) is linear: fold it into one [192, 128] matrix.
    W_mo_r = w["W_mo"].reshape(128, H, D)                       # [128, H, 64]
    F = jnp.einsum("ed,mhe->hdm", w["W_cv"], W_mo_r)            # [H, 64, 128]
    F2v = w["W_v"].T @ F[2]                                     # [64, 128]
    Fall = jnp.concatenate([F[0], F[1], F2v], axis=0)           # [192, 128]

    concat2 = jnp.concatenate([own_e, env_e, pooled], axis=1)   # [b, 192]
    mo = concat2 @ Fall + w["b_mo"]                             # [b, 128]
    a = jax.nn.relu(mo @ w["W_a1"].T + w["b_a1"])               # [b, 64]
    out = jnp.tanh(a @ w["W_a2"].T + w["b_a2"])                 # [b, NACT]
    return out[:, None, :]                                      # [b, 1, NACT]


_pmapped = jax.pmap(_forward, in_axes=(0, 0, 0, None))

_compiled = False


def kernel(**inputs) -> np.ndarray:
    global _compiled
    s0 = np.asarray(inputs["s0"], dtype=np.float32).reshape(M, B // M, -1)
    s1 = np.asarray(inputs["s1"], dtype=np.float32).reshape(M, B // M, -1)
    s2 = np.asarray(inputs["s2"], dtype=np.float32).reshape(M, B // M, N, -1)
    w = {
        k: jnp.asarray(np.asarray(v, dtype=np.float32))
        for k, v in inputs.items()
        if k not in ("s0", "s1", "s2")
    }
    out = _pmapped(jnp.asarray(s0), jnp.asarray(s1), jnp.asarray(s2), w)
    out = np.asarray(jax.device_get(out), dtype=np.float32)
    return out.reshape(B, 1, NACT)
